# revision 39
# baseline (speedup 1.0000x reference)
"""Trainium2 Bass kernel for the NodeAttentionLayer (GAT-style) problem.

Math (per reference.py):
    h_t = t_input @ W_t; h_o = o_input @ W_o
    s_t = h_t @ a[:F];  s_o = h_o @ a[F:]
    e[i,j]   = leaky_relu(s_t[i] + s_o[j], 0.2)
    att      = softmax(where(adj>0, e, -9e15), axis=1)
    out      = elu(att @ h_o)

Single-mask identity used on-device:
    exp(lrelu(y)) = exp(0.2 y) * max(exp(0.8 y), 1),   y = s_t[i] + s_o[j]
The i-side factor exp(0.2 s_t) cancels in the softmax ratio, so with
    q[j,i] = max(exp(0.8 s_t_i) * exp(s_o_j), exp(0.2 s_o_j))
           = exp(0.2 s_o_j) * max(exp(0.8 y), 1)
the (unnormalized, u2-cancelled) attention is m = q * adj and
    out[:,i] = elu( (W_ext^T m)[0:F,i] / (W_ext^T m)[F,i] ),  W_ext = [h_o | 1].
q is ONE dual-op tensor_scalar (mult,max with two per-partition scalar
columns) and m ONE tensor_tensor mult per j-tile — both on DVE in 2x/4x
modes.  One matmul pair per tile accumulates in PSUM over all 64 tiles
with a fixed 65-column stationary per tile.  GpSimd is untouched (its
SBUF port contention slows DVE ~4x).

h_t is never materialized: s_t = t_input @ (W_t a_t) is a matvec, and
s_o rides as a 65th projection column ([W_o | W_o a_o] moving operand).

Sharding: rows of t_input/adj (N_t) split across 8 cores; o replicated.
Kernel computes output TRANSPOSED ([F, rows]) per core; host transposes.
adj fed per-core as adj[rows,:].T in bf16 (0/1 -> lossless); t/o in bf16.
"""

import contextlib
import ctypes
import sys
import tempfile
import types

import ml_dtypes
import numpy as np

import concourse.bass as bass
import concourse.mybir as mybir
import concourse.tile as tile
from concourse.vector_clock import ScopedClock

bf16 = ml_dtypes.bfloat16

# ---------------------------------------------------------------------------
# Environment shims
# ---------------------------------------------------------------------------

def _patch_tile_drain():
    """walrus in this container allows only one sync-wait per sync-engine
    instruction; split the TileContext epilogue drain's waits onto
    individual nops."""
    if getattr(tile.TileContext, "_drain_patch_installed", False):
        return

    def _drain_and_barrier(self, tick_clock, wait_clock):
        nop_inst = self.nc.sync.nop(nofuse=True)
        wait_clock.add_sem_waits(
            nop_inst.ins, ScopedClock({None: tick_clock.global_clock})
        )
        ow = list(nop_inst.ins.sync_info.on_wait) if nop_inst.ins.sync_info else []
        if len(ow) > 1:
            nop_inst.ins.sync_info.on_wait = ow[:1]
            for w in ow[1:]:
                extra = self.nc.sync.nop(nofuse=True)
                if extra.ins.sync_info is None:
                    extra.ins.sync_info = mybir.SyncInfo(on_wait=[w], on_update=[])
                else:
                    extra.ins.sync_info.on_wait = [w]
        self.nc.sync.drain()
        self.nc.all_engine_barrier()
        popped = self.nc._tile_sem_poison_stack.pop()
        assert popped is self._sem_poison
        self.nc.clear_and_free_semaphores(list(self.sems.allocated().values()))
        self.nc.all_engine_barrier()

    tile.TileContext._drain_and_barrier = _drain_and_barrier
    tile.TileContext._drain_patch_installed = True


def _install_ntff_hook():
    """Provide antenv.axon_hooks (absent in this image) so trace=True works."""
    if "antenv.axon_hooks" in sys.modules:
        return
    import antenv

    state = {"hook": None}
    mod = types.ModuleType("antenv.axon_hooks")
    mod.set_axon_ntff_profile_hook = lambda h: state.__setitem__("hook", h)
    mod.get_axon_ntff_profile_hook = lambda: state["hook"]
    sys.modules["antenv.axon_hooks"] = mod
    antenv.axon_hooks = mod

    try:
        lib = ctypes.CDLL("/opt/axon/libaxon_pjrt.so")
    except OSError:
        return
    if not hasattr(lib, "axon_start_nrt_profile"):
        return
    lib.axon_start_nrt_profile.argtypes = [
        ctypes.POINTER(ctypes.c_int64),
        ctypes.c_size_t,
    ]
    lib.axon_start_nrt_profile.restype = ctypes.c_int64
    lib.axon_stop_nrt_profile.argtypes = [ctypes.c_char_p]
    lib.axon_stop_nrt_profile.restype = ctypes.c_int64

    @contextlib.contextmanager
    def _ntff_hook(output_dir, device_ids):
        import jax

        jax.devices()
        if device_ids:
            ids = (ctypes.c_int64 * len(device_ids))(*device_ids)
            rc = lib.axon_start_nrt_profile(ids, len(device_ids))
        else:
            rc = lib.axon_start_nrt_profile(None, 0)
        if rc != 0:
            raise RuntimeError(f"axon_start_nrt_profile rc={rc}")
        try:
            yield
        finally:
            n = lib.axon_stop_nrt_profile(str(output_dir).encode())
            print(f"profile: {n} file(s) written to {output_dir}", file=sys.stderr)

    state["hook"] = _ntff_hook


_patch_tile_drain()
_install_ntff_hook()


def _split_multi_waits(nc):
    """walrus here accepts at most ONE sync-wait per instruction; hoist extra
    waits onto same-engine nops inserted immediately before."""
    import bass_rust

    k = 0
    for f in nc.m.functions:
        for blk in f.blocks:
            insts = blk.instructions
            out = []
            changed = False
            for inst in insts:
                si = inst.sync_info
                ow = list(si.on_wait) if si is not None else []
                if len(ow) > 1:
                    for w in ow[:-1]:
                        nop = bass_rust.InstNoOp(
                            name=f"waitsplit-{k}", engine=inst.engine
                        )
                        k += 1
                        nop.sync_info = mybir.SyncInfo(on_wait=[w], on_update=[])
                        out.append(nop)
                    si.on_wait = [ow[-1]]
                    changed = True
                out.append(inst)
            if changed:
                blk.instructions = out

# ---------------------------------------------------------------------------
# Problem constants (hardcoded per spec)
# ---------------------------------------------------------------------------
N_T, N_O, F_IN, F_OUT = 8192, 8192, 256, 64
N_CORES = 8
R = N_T // N_CORES            # rows (i) per core = 1024
NJ = N_O // 128               # j tiles of 128 = 64
KC = F_IN // 128              # contraction chunks for projections = 2
GROUP = 16                    # j-tiles per setup group
NG = NJ // GROUP              # setup groups = 4
FX = F_OUT + 1                # 65 (h_o columns + ones/denominator column)
F32 = mybir.dt.float32
BF16 = mybir.dt.bfloat16
AF = mybir.ActivationFunctionType
OP = mybir.AluOpType


def _ap(tensor, offset, ap):
    return bass.AP(tensor=tensor, offset=offset, ap=ap)


def _act_recip(nc, out, in_):
    """Raw InstActivation(Reciprocal) — table-interp reciprocal on the Scalar
    engine.  bass.activation() refuses Reciprocal on accuracy grounds; our
    denominators are well-conditioned sums of positive terms and the output
    gate is 2e-2, so the table version is fine (validated vs reference)."""
    eng = nc.scalar
    inputs = [eng.lower_ap(in_)]
    for v in (0.0, 1.0, 0.0):  # bias, scale, alpha
        inputs.append(mybir.ImmediateValue(dtype=mybir.dt.float32, value=v))
    return eng.add_instruction(
        mybir.InstActivation(
            name=eng.bass.get_next_instruction_name(),
            func=AF.Reciprocal,
            ins=inputs,
            outs=[eng.lower_ap(out)],
        )
    )


def build_kernel(split_waits=True):
    nc = bass.Bass("TRN2")

    t_T = nc.dram_tensor("t_T", [F_IN, R], BF16, kind="ExternalInput")
    o_T = nc.dram_tensor("o_T", [F_IN, N_O], BF16, kind="ExternalInput")
    w_to = nc.dram_tensor("w_to", [F_IN, 2 * F_OUT], F32, kind="ExternalInput")
    a_vec = nc.dram_tensor("a_vec", [2 * F_OUT, 1], F32, kind="ExternalInput")
    adjT = nc.dram_tensor("adjT", [N_O, R], BF16, kind="ExternalInput")
    out = nc.dram_tensor("out", [F_OUT, R], BF16, kind="ExternalOutput")

    with tile.TileContext(nc) as tc, contextlib.ExitStack() as ctx:
        adj_pool = ctx.enter_context(tc.tile_pool(name="adj", bufs=5))
        q_pool = ctx.enter_context(tc.tile_pool(name="q", bufs=6))
        m_pool = ctx.enter_context(tc.tile_pool(name="m", bufs=6))
        singles = ctx.enter_context(tc.tile_pool(name="singles", bufs=1))
        stage = ctx.enter_context(tc.tile_pool(name="stage", bufs=2))
        acc_psum = ctx.enter_context(tc.tile_pool(name="acc", bufs=1, space="PSUM"))
        misc_psum = ctx.enter_context(tc.tile_pool(name="mpsum", bufs=2, space="PSUM"))

        # ------------------------------------------------------------------
        # Weights + a-vector broadcasts (small loads issued from the
        # otherwise-idle GpSimd/Scalar sequencers so they don't queue behind
        # the big sync-engine DMA stream)
        # ------------------------------------------------------------------
        w_sb = singles.tile([128, KC, 2 * F_OUT], F32)
        nc.sync.dma_start(
            out=w_sb[:, :, :],
            in_=_ap(w_to, 0, [[2 * F_OUT, 128], [128 * 2 * F_OUT, KC], [1, 2 * F_OUT]]),
        )
        at_b = singles.tile([128, F_OUT], F32)
        ao_b = singles.tile([128, F_OUT], F32)
        nc.sync.dma_start(out=at_b[:, :], in_=_ap(a_vec, 0, [[0, 128], [1, F_OUT]]))
        nc.scalar.dma_start(
            out=ao_b[:, :], in_=_ap(a_vec, F_OUT, [[0, 128], [1, F_OUT]])
        )
        t_sb = singles.tile([128, KC, R], BF16)
        nc.scalar.dma_start(
            out=t_sb[:, :, :], in_=_ap(t_T, 0, [[R, 128], [128 * R, KC], [1, R]])
        )

        # wta = W_t @ a_t replicated to a [128] stationary block (so the s_t
        # matmul broadcasts s_t to all partitions); woa = W_o @ a_o (65th col)
        prod_t = stage.tile([128, KC, F_OUT], F32, tag="prod")
        nc.vector.tensor_tensor(
            prod_t[:, :, :],
            w_sb[:, :, 0:F_OUT],
            _ap(at_b[:, :].tensor, at_b[:, :].offset,
                [at_b[:, :].ap[0], [0, KC], [1, F_OUT]]),
            OP.mult,
        )
        red_t = stage.tile([128, KC], F32, tag="red")
        nc.vector.tensor_reduce(red_t[:, :], prod_t[:, :, :], mybir.AxisListType.X, OP.add)
        wta_rep = singles.tile([128, KC, 128], BF16)
        nc.vector.tensor_copy(
            wta_rep[:, :, :],
            _ap(red_t[:, :].tensor, red_t[:, :].offset,
                [red_t[:, :].ap[0], [1, KC], [0, 128]]),
        )

        prod_o = stage.tile([128, KC, F_OUT], F32, tag="prod")
        nc.vector.tensor_tensor(
            prod_o[:, :, :],
            w_sb[:, :, F_OUT:2 * F_OUT],
            _ap(ao_b[:, :].tensor, ao_b[:, :].offset,
                [ao_b[:, :].ap[0], [0, KC], [1, F_OUT]]),
            OP.mult,
        )
        woa_f = stage.tile([128, KC], F32, tag="red")
        nc.vector.tensor_reduce(woa_f[:, :], prod_o[:, :, :], mybir.AxisListType.X, OP.add)

        # moving operand for the o-projection: [W_o | W_o a_o] in bf16
        woe = singles.tile([128, KC, FX], BF16)
        nc.vector.tensor_copy(woe[:, :, 0:F_OUT], w_sb[:, :, F_OUT:2 * F_OUT])
        nc.vector.tensor_copy(woe[:, :, F_OUT], woa_f[:, :])

        # ------------------------------------------------------------------
        # o side, graded groups (small first group so the main loop starts
        # early): h_o_ext = [h_o | s_o] -> hoe=[h_o|1], v1, v2
        # ------------------------------------------------------------------
        GROUPS = [(0, 16), (16, 16), (32, 16), (48, 16)]
        tile2grp = {}
        for gi, (t0, nt) in enumerate(GROUPS):
            for u in range(nt):
                tile2grp[t0 + u] = (gi, u)

        hoe, v1g, v2g = [], [], []

        def o_group(gi):
            t0, nt = GROUPS[gi]
            cols = nt * 128
            o_sb = stage.tile([128, KC, GROUP * 128], BF16, tag="osb")
            for c in range(KC):
                for h0 in range(0, cols, 1024):
                    h1 = min(h0 + 1024, cols)
                    nc.sync.dma_start(
                        out=o_sb[:, c, h0:h1],
                        in_=o_T[c * 128:(c + 1) * 128,
                                t0 * 128 + h0:t0 * 128 + h1],
                    )
            hoe_g = singles.tile([128, GROUP, FX], BF16, tag=f"hoe{gi}")
            v1_g = singles.tile([128, GROUP], F32, tag=f"v1{gi}")
            v2_g = singles.tile([128, GROUP], F32, tag=f"v2{gi}")
            for b in range(nt // 4):
                ho_ps = misc_psum.tile([128, 4, FX], F32, tag="mps")
                for s in range(4):
                    j0 = (b * 4 + s) * 128
                    for c in range(KC):
                        nc.tensor.matmul(
                            ho_ps[:, s, :],
                            o_sb[:, c, j0:j0 + 128],
                            woe[:, c, :],
                            start=(c == 0),
                            stop=(c == KC - 1),
                        )
                sl = slice(b * 4, b * 4 + 4)
                nc.scalar.activation(v1_g[:, sl], ho_ps[:, :, F_OUT], AF.Exp)
                nc.scalar.activation(
                    v2_g[:, sl], ho_ps[:, :, F_OUT], AF.Exp, scale=0.2
                )
                nc.scalar.activation(hoe_g[:, sl, 0:F_OUT], ho_ps[:, :, 0:F_OUT], AF.Copy)
                nc.vector.memset(hoe_g[:, sl, F_OUT], 1.0)
            hoe.append(hoe_g)
            v1g.append(v1_g)
            v2g.append(v2_g)

        o_group(0)
        o_group(1)

        # ------------------------------------------------------------------
        # t side: s_t broadcast to all 128 partitions directly by the matmul
        # (stationary = wta replicated 128x); u8 = exp(0.8 s_t) on Act.
        # ------------------------------------------------------------------
        u8_b = singles.tile([128, R], BF16)
        with tc.high_priority():
            for n in range(R // 512):
                sl = slice(n * 512, (n + 1) * 512)
                st_ps = misc_psum.tile([128, 512], F32, tag="stps")
                for c in range(KC):
                    nc.tensor.matmul(
                        st_ps[:, :],
                        wta_rep[:, c, :],
                        t_sb[:, c, sl],
                        start=(c == 0),
                        stop=(c == KC - 1),
                    )
                nc.scalar.activation(u8_b[:, sl], st_ps[:, :], AF.Exp, scale=0.8)

        o_group(1)

        # ------------------------------------------------------------------
        # Main loop over j tiles: q = max(u8*v1, v2); m = q*adj; accumulate
        # adj DMA'd in 4-tile batches (fewer descriptors/semaphores).
        # ------------------------------------------------------------------
        acc = [
            acc_psum.tile([FX, 512], F32, tag=f"acc{n}", name=f"acc{n}")
            for n in range(2)
        ]
        for t in range(NJ):
            g, u = tile2grp[t]
            # emit later o-group setup just-in-time so its DMA doesn't
            # compete with the adj stream at loop start
            if t == 10:
                o_group(2)
            elif t == 30:
                o_group(3)
            adj_t = adj_pool.tile([128, R], BF16)
            nc.sync.dma_start(out=adj_t[:, :], in_=adjT[t * 128:(t + 1) * 128, :])
            q_t = q_pool.tile([128, R], BF16)
            nc.vector.tensor_scalar(
                q_t[:, :], u8_b[:, :], v1g[g][:, u:u + 1], v2g[g][:, u:u + 1],
                OP.mult, OP.max,
            )
            m_t = m_pool.tile([128, R], BF16)
            nc.vector.tensor_tensor(m_t[:, :], q_t[:, :], adj_t[:, :], OP.mult)
            for n in range(2):
                nc.tensor.matmul(
                    acc[n][:, :],
                    hoe[g][:, u, :],
                    m_t[:, n * 512:(n + 1) * 512],
                    start=(t == 0),
                    stop=(t == NJ - 1),
                )

        # ------------------------------------------------------------------
        # Tail: out = elu(T[0:F]/T[F]), 2 pipelined 512-chunks.
        # Reciprocal on the Scalar engine (both halves, one table load),
        # denominator broadcast via ones-column matmul, elu as
        # max(x,0) + min(exp(x)-1, 0) so the Act exp overlaps DVE.
        # ------------------------------------------------------------------
        ones_row = singles.tile([1, F_OUT], BF16)
        nc.vector.memset(ones_row[:, :], 1.0)
        zr_row = singles.tile([1, R], BF16)
        h_c = singles.tile([F_OUT, R], F32)
        ot = singles.tile([F_OUT, R], F32)
        ex_sb = singles.tile([F_OUT, R], F32)
        ob = singles.tile([F_OUT, R], BF16)
        for n in range(2):
            sl = slice(n * 512, (n + 1) * 512)
            _act_recip(nc, zr_row[:, sl], acc[n][F_OUT:FX, :])
            nc.vector.tensor_copy(h_c[:, sl], acc[n][0:F_OUT, :])
        for n in range(2):
            sl = slice(n * 512, (n + 1) * 512)
            zb_ps = misc_psum.tile([F_OUT, 512], F32, tag="mps")
            nc.tensor.matmul(
                zb_ps[:, :], ones_row[:, :], zr_row[:, sl], start=True, stop=True
            )
            nc.vector.tensor_tensor(ot[:, sl], h_c[:, sl], zb_ps[:, :], OP.mult)
            nc.scalar.activation(ex_sb[:, sl], ot[:, sl], AF.Exp)
            nc.vector.tensor_scalar(ot[:, sl], ot[:, sl], 0.0, None, OP.max)
            nc.vector.tensor_scalar(
                ex_sb[:, sl], ex_sb[:, sl], -1.0, 0.0, OP.add, OP.min
            )
            nc.vector.tensor_tensor(ob[:, sl], ot[:, sl], ex_sb[:, sl], OP.add)
            nc.sync.dma_start(out=out[:, sl], in_=ob[:, sl])

    if split_waits:
        _split_multi_waits(nc)
    return nc


_CACHED = {}


def _get_compiled():
    if "nc" not in _CACHED:
        _CACHED["nc"] = build_kernel()
    return _CACHED["nc"]


def kernel(t_input, o_input, W_t, W_o, a, adj, _trace=False):
    from concourse.bass_utils import run_bass_kernel_spmd

    t_input = np.asarray(t_input, dtype=np.float32)
    o_input = np.asarray(o_input, dtype=np.float32)
    W_t = np.asarray(W_t, dtype=np.float32)
    W_o = np.asarray(W_o, dtype=np.float32)
    a = np.asarray(a, dtype=np.float32)
    adj = np.asarray(adj)

    o_T = np.ascontiguousarray(o_input.T).astype(bf16)
    adj_b = adj.astype(bf16)
    w_to = np.ascontiguousarray(np.concatenate([W_t, W_o], axis=1))

    in_maps = []
    for m in range(N_CORES):
        rows = slice(m * R, (m + 1) * R)
        in_maps.append(
            {
                "t_T": np.ascontiguousarray(t_input[rows, :].T).astype(bf16),
                "o_T": o_T,
                "w_to": w_to,
                "a_vec": a,
                "adjT": np.ascontiguousarray(adj_b[rows, :].T),
            }
        )

    nc = _get_compiled()
    res = run_bass_kernel_spmd(
        nc, in_maps, core_ids=list(range(N_CORES)), trace=_trace
    )
    out = np.empty((N_T, F_OUT), dtype=np.float32)
    for m in range(N_CORES):
        out[m * R:(m + 1) * R, :] = res.results[m]["out"].T.astype(np.float32)
    if _trace:
        kernel.last_exec_time_ns = res.exec_time_ns
        kernel.last_results = res
    return out


# revision 40
# speedup vs baseline: 1.1034x; 1.1034x over previous
"""Trainium2 Bass kernel for the NodeAttentionLayer (GAT-style) problem.

Math (per reference.py):
    h_t = t_input @ W_t; h_o = o_input @ W_o
    s_t = h_t @ a[:F];  s_o = h_o @ a[F:]
    e[i,j]   = leaky_relu(s_t[i] + s_o[j], 0.2)
    att      = softmax(where(adj>0, e, -9e15), axis=1)
    out      = elu(att @ h_o)

Single-mask identity used on-device:
    exp(lrelu(y)) = exp(0.2 y) * max(exp(0.8 y), 1),   y = s_t[i] + s_o[j]
The i-side factor exp(0.2 s_t) cancels in the softmax ratio, so with
    q[j,i] = max(exp(0.8 s_t_i) * exp(s_o_j), exp(0.2 s_o_j))
           = exp(0.2 s_o_j) * max(exp(0.8 y), 1)
the (unnormalized, u2-cancelled) attention is m = q * adj and
    out[:,i] = elu( (W_ext^T m)[0:F,i] / (W_ext^T m)[F,i] ),  W_ext = [h_o | 1].
q is ONE dual-op tensor_scalar (mult,max with two per-partition scalar
columns) and m ONE tensor_tensor mult per j-tile — both on DVE in 2x/4x
modes.  One matmul pair per tile accumulates in PSUM over all 64 tiles
with a fixed 65-column stationary per tile.  GpSimd is untouched (its
SBUF port contention slows DVE ~4x).

h_t is never materialized: s_t = t_input @ (W_t a_t) is a matvec, and
s_o rides as a 65th projection column ([W_o | W_o a_o] moving operand).

Sharding: rows of t_input/adj (N_t) split across 8 cores; o replicated.
Kernel computes output TRANSPOSED ([F, rows]) per core; host transposes.
adj fed per-core as adj[rows,:].T in bf16 (0/1 -> lossless); t/o in bf16.
"""

import contextlib
import ctypes
import sys
import tempfile
import types

import ml_dtypes
import numpy as np

import concourse.bass as bass
import concourse.mybir as mybir
import concourse.tile as tile
from concourse.vector_clock import ScopedClock

bf16 = ml_dtypes.bfloat16

# ---------------------------------------------------------------------------
# Environment shims
# ---------------------------------------------------------------------------

def _patch_tile_drain():
    """walrus in this container allows only one sync-wait per sync-engine
    instruction; split the TileContext epilogue drain's waits onto
    individual nops."""
    if getattr(tile.TileContext, "_drain_patch_installed", False):
        return

    def _drain_and_barrier(self, tick_clock, wait_clock):
        nop_inst = self.nc.sync.nop(nofuse=True)
        wait_clock.add_sem_waits(
            nop_inst.ins, ScopedClock({None: tick_clock.global_clock})
        )
        ow = list(nop_inst.ins.sync_info.on_wait) if nop_inst.ins.sync_info else []
        if len(ow) > 1:
            nop_inst.ins.sync_info.on_wait = ow[:1]
            for w in ow[1:]:
                extra = self.nc.sync.nop(nofuse=True)
                if extra.ins.sync_info is None:
                    extra.ins.sync_info = mybir.SyncInfo(on_wait=[w], on_update=[])
                else:
                    extra.ins.sync_info.on_wait = [w]
        self.nc.sync.drain()
        self.nc.all_engine_barrier()
        popped = self.nc._tile_sem_poison_stack.pop()
        assert popped is self._sem_poison
        self.nc.clear_and_free_semaphores(list(self.sems.allocated().values()))
        self.nc.all_engine_barrier()

    tile.TileContext._drain_and_barrier = _drain_and_barrier
    tile.TileContext._drain_patch_installed = True


def _install_ntff_hook():
    """Provide antenv.axon_hooks (absent in this image) so trace=True works."""
    if "antenv.axon_hooks" in sys.modules:
        return
    import antenv

    state = {"hook": None}
    mod = types.ModuleType("antenv.axon_hooks")
    mod.set_axon_ntff_profile_hook = lambda h: state.__setitem__("hook", h)
    mod.get_axon_ntff_profile_hook = lambda: state["hook"]
    sys.modules["antenv.axon_hooks"] = mod
    antenv.axon_hooks = mod

    try:
        lib = ctypes.CDLL("/opt/axon/libaxon_pjrt.so")
    except OSError:
        return
    if not hasattr(lib, "axon_start_nrt_profile"):
        return
    lib.axon_start_nrt_profile.argtypes = [
        ctypes.POINTER(ctypes.c_int64),
        ctypes.c_size_t,
    ]
    lib.axon_start_nrt_profile.restype = ctypes.c_int64
    lib.axon_stop_nrt_profile.argtypes = [ctypes.c_char_p]
    lib.axon_stop_nrt_profile.restype = ctypes.c_int64

    @contextlib.contextmanager
    def _ntff_hook(output_dir, device_ids):
        import jax

        jax.devices()
        if device_ids:
            ids = (ctypes.c_int64 * len(device_ids))(*device_ids)
            rc = lib.axon_start_nrt_profile(ids, len(device_ids))
        else:
            rc = lib.axon_start_nrt_profile(None, 0)
        if rc != 0:
            raise RuntimeError(f"axon_start_nrt_profile rc={rc}")
        try:
            yield
        finally:
            n = lib.axon_stop_nrt_profile(str(output_dir).encode())
            print(f"profile: {n} file(s) written to {output_dir}", file=sys.stderr)

    state["hook"] = _ntff_hook


_patch_tile_drain()
_install_ntff_hook()


def _split_multi_waits(nc):
    """walrus here accepts at most ONE sync-wait per instruction; hoist extra
    waits onto same-engine nops inserted immediately before."""
    import bass_rust

    k = 0
    for f in nc.m.functions:
        for blk in f.blocks:
            insts = blk.instructions
            out = []
            changed = False
            for inst in insts:
                si = inst.sync_info
                ow = list(si.on_wait) if si is not None else []
                if len(ow) > 1:
                    for w in ow[:-1]:
                        nop = bass_rust.InstNoOp(
                            name=f"waitsplit-{k}", engine=inst.engine
                        )
                        k += 1
                        nop.sync_info = mybir.SyncInfo(on_wait=[w], on_update=[])
                        out.append(nop)
                    si.on_wait = [ow[-1]]
                    changed = True
                out.append(inst)
            if changed:
                blk.instructions = out

# ---------------------------------------------------------------------------
# Problem constants (hardcoded per spec)
# ---------------------------------------------------------------------------
N_T, N_O, F_IN, F_OUT = 8192, 8192, 256, 64
N_CORES = 8
R = N_T // N_CORES            # rows (i) per core = 1024
NJ = N_O // 128               # j tiles of 128 = 64
KC = F_IN // 128              # contraction chunks for projections = 2
GROUP = 16                    # j-tiles per setup group
NG = NJ // GROUP              # setup groups = 4
FX = F_OUT + 1                # 65 (h_o columns + ones/denominator column)
F32 = mybir.dt.float32
BF16 = mybir.dt.bfloat16
AF = mybir.ActivationFunctionType
OP = mybir.AluOpType


def _ap(tensor, offset, ap):
    return bass.AP(tensor=tensor, offset=offset, ap=ap)


def _act_recip(nc, out, in_):
    """Raw InstActivation(Reciprocal) — table-interp reciprocal on the Scalar
    engine.  bass.activation() refuses Reciprocal on accuracy grounds; our
    denominators are well-conditioned sums of positive terms and the output
    gate is 2e-2, so the table version is fine (validated vs reference)."""
    eng = nc.scalar
    inputs = [eng.lower_ap(in_)]
    for v in (0.0, 1.0, 0.0):  # bias, scale, alpha
        inputs.append(mybir.ImmediateValue(dtype=mybir.dt.float32, value=v))
    return eng.add_instruction(
        mybir.InstActivation(
            name=eng.bass.get_next_instruction_name(),
            func=AF.Reciprocal,
            ins=inputs,
            outs=[eng.lower_ap(out)],
        )
    )


def build_kernel(split_waits=True):
    nc = bass.Bass("TRN2")

    t_T = nc.dram_tensor("t_T", [F_IN, R], BF16, kind="ExternalInput")
    o_T = nc.dram_tensor("o_T", [F_IN, N_O], BF16, kind="ExternalInput")
    w_to = nc.dram_tensor("w_to", [F_IN, 2 * F_OUT], F32, kind="ExternalInput")
    a_vec = nc.dram_tensor("a_vec", [2 * F_OUT, 1], F32, kind="ExternalInput")
    adjT = nc.dram_tensor("adjT", [N_O, R], BF16, kind="ExternalInput")
    out = nc.dram_tensor("out", [F_OUT, R], BF16, kind="ExternalOutput")

    with tile.TileContext(nc) as tc, contextlib.ExitStack() as ctx:
        adj_pool = ctx.enter_context(tc.tile_pool(name="adj", bufs=8))
        q_pool = ctx.enter_context(tc.tile_pool(name="q", bufs=6))
        m_pool = ctx.enter_context(tc.tile_pool(name="m", bufs=6))
        singles = ctx.enter_context(tc.tile_pool(name="singles", bufs=1))
        stage = ctx.enter_context(tc.tile_pool(name="stage", bufs=2))
        acc_psum = ctx.enter_context(tc.tile_pool(name="acc", bufs=1, space="PSUM"))
        misc_psum = ctx.enter_context(tc.tile_pool(name="mpsum", bufs=2, space="PSUM"))

        # ------------------------------------------------------------------
        # Weights + a-vector broadcasts (small loads issued from the
        # otherwise-idle GpSimd/Scalar sequencers so they don't queue behind
        # the big sync-engine DMA stream)
        # ------------------------------------------------------------------
        w_sb = singles.tile([128, KC, 2 * F_OUT], F32)
        nc.sync.dma_start(
            out=w_sb[:, :, :],
            in_=_ap(w_to, 0, [[2 * F_OUT, 128], [128 * 2 * F_OUT, KC], [1, 2 * F_OUT]]),
        )
        at_b = singles.tile([128, F_OUT], F32)
        ao_b = singles.tile([128, F_OUT], F32)
        nc.sync.dma_start(out=at_b[:, :], in_=_ap(a_vec, 0, [[0, 128], [1, F_OUT]]))
        nc.scalar.dma_start(
            out=ao_b[:, :], in_=_ap(a_vec, F_OUT, [[0, 128], [1, F_OUT]])
        )
        t_sb = singles.tile([128, KC, R], BF16)
        nc.scalar.dma_start(
            out=t_sb[:, :, :], in_=_ap(t_T, 0, [[R, 128], [128 * R, KC], [1, R]])
        )

        # wta = W_t @ a_t replicated to a [128] stationary block (so the s_t
        # matmul broadcasts s_t to all partitions); woa = W_o @ a_o (65th col)
        prod_t = stage.tile([128, KC, F_OUT], F32, tag="prod")
        nc.vector.tensor_tensor(
            prod_t[:, :, :],
            w_sb[:, :, 0:F_OUT],
            _ap(at_b[:, :].tensor, at_b[:, :].offset,
                [at_b[:, :].ap[0], [0, KC], [1, F_OUT]]),
            OP.mult,
        )
        red_t = stage.tile([128, KC], F32, tag="red")
        nc.vector.tensor_reduce(red_t[:, :], prod_t[:, :, :], mybir.AxisListType.X, OP.add)
        wta_rep = singles.tile([128, KC, 128], BF16)
        nc.vector.tensor_copy(
            wta_rep[:, :, :],
            _ap(red_t[:, :].tensor, red_t[:, :].offset,
                [red_t[:, :].ap[0], [1, KC], [0, 128]]),
        )

        prod_o = stage.tile([128, KC, F_OUT], F32, tag="prod")
        nc.vector.tensor_tensor(
            prod_o[:, :, :],
            w_sb[:, :, F_OUT:2 * F_OUT],
            _ap(ao_b[:, :].tensor, ao_b[:, :].offset,
                [ao_b[:, :].ap[0], [0, KC], [1, F_OUT]]),
            OP.mult,
        )
        woa_f = stage.tile([128, KC], F32, tag="red")
        nc.vector.tensor_reduce(woa_f[:, :], prod_o[:, :, :], mybir.AxisListType.X, OP.add)

        # moving operand for the o-projection: [W_o | W_o a_o] in bf16
        woe = singles.tile([128, KC, FX], BF16)
        nc.vector.tensor_copy(woe[:, :, 0:F_OUT], w_sb[:, :, F_OUT:2 * F_OUT])
        nc.vector.tensor_copy(woe[:, :, F_OUT], woa_f[:, :])

        # ------------------------------------------------------------------
        # o side, graded groups (small first group so the main loop starts
        # early): h_o_ext = [h_o | s_o] -> hoe=[h_o|1], v1, v2
        # ------------------------------------------------------------------
        GROUPS = [(0, 16), (16, 16), (32, 16), (48, 16)]
        tile2grp = {}
        for gi, (t0, nt) in enumerate(GROUPS):
            for u in range(nt):
                tile2grp[t0 + u] = (gi, u)

        hoe, v1g, v2g = [], [], []

        def o_group(gi):
            t0, nt = GROUPS[gi]
            cols = nt * 128
            o_sb = stage.tile([128, KC, GROUP * 128], BF16, tag="osb")
            for c in range(KC):
                for h0 in range(0, cols, 1024):
                    h1 = min(h0 + 1024, cols)
                    nc.sync.dma_start(
                        out=o_sb[:, c, h0:h1],
                        in_=o_T[c * 128:(c + 1) * 128,
                                t0 * 128 + h0:t0 * 128 + h1],
                    )
            hoe_g = singles.tile([128, GROUP, FX], BF16, tag=f"hoe{gi}")
            v1_g = singles.tile([128, GROUP], F32, tag=f"v1{gi}")
            v2_g = singles.tile([128, GROUP], F32, tag=f"v2{gi}")
            for b in range(nt // 4):
                ho_ps = misc_psum.tile([128, 4, FX], F32, tag="mps")
                for s in range(4):
                    j0 = (b * 4 + s) * 128
                    for c in range(KC):
                        nc.tensor.matmul(
                            ho_ps[:, s, :],
                            o_sb[:, c, j0:j0 + 128],
                            woe[:, c, :],
                            start=(c == 0),
                            stop=(c == KC - 1),
                        )
                sl = slice(b * 4, b * 4 + 4)
                nc.scalar.activation(v1_g[:, sl], ho_ps[:, :, F_OUT], AF.Exp)
                nc.scalar.activation(
                    v2_g[:, sl], ho_ps[:, :, F_OUT], AF.Exp, scale=0.2
                )
                nc.scalar.activation(hoe_g[:, sl, 0:F_OUT], ho_ps[:, :, 0:F_OUT], AF.Copy)
                nc.vector.memset(hoe_g[:, sl, F_OUT], 1.0)
            hoe.append(hoe_g)
            v1g.append(v1_g)
            v2g.append(v2_g)

        o_group(0)
        o_group(1)

        # ------------------------------------------------------------------
        # t side: s_t broadcast to all 128 partitions directly by the matmul
        # (stationary = wta replicated 128x); u8 = exp(0.8 s_t) on Act.
        # ------------------------------------------------------------------
        u8_b = singles.tile([128, R], BF16)
        with tc.high_priority():
            for n in range(R // 512):
                sl = slice(n * 512, (n + 1) * 512)
                st_ps = misc_psum.tile([128, 512], F32, tag="stps")
                for c in range(KC):
                    nc.tensor.matmul(
                        st_ps[:, :],
                        wta_rep[:, c, :],
                        t_sb[:, c, sl],
                        start=(c == 0),
                        stop=(c == KC - 1),
                    )
                nc.scalar.activation(u8_b[:, sl], st_ps[:, :], AF.Exp, scale=0.8)

        o_group(1)

        # ------------------------------------------------------------------
        # Main loop over j tiles: q = max(u8*v1, v2); m = q*adj; accumulate
        # adj DMA'd in 4-tile batches (fewer descriptors/semaphores).
        # ------------------------------------------------------------------
        acc = [
            acc_psum.tile([FX, 512], F32, tag=f"acc{n}", name=f"acc{n}")
            for n in range(2)
        ]
        for t in range(NJ):
            g, u = tile2grp[t]
            # emit later o-group setup just-in-time so its DMA doesn't
            # compete with the adj stream at loop start
            if t == 6:
                o_group(2)
            elif t == 22:
                o_group(3)
            adj_t = adj_pool.tile([128, R], BF16)
            nc.sync.dma_start(out=adj_t[:, :], in_=adjT[t * 128:(t + 1) * 128, :])
            q_t = q_pool.tile([128, R], BF16)
            nc.vector.tensor_scalar(
                q_t[:, :], u8_b[:, :], v1g[g][:, u:u + 1], v2g[g][:, u:u + 1],
                OP.mult, OP.max,
            )
            m_t = m_pool.tile([128, R], BF16)
            nc.vector.tensor_tensor(m_t[:, :], q_t[:, :], adj_t[:, :], OP.mult)
            for n in range(2):
                nc.tensor.matmul(
                    acc[n][:, :],
                    hoe[g][:, u, :],
                    m_t[:, n * 512:(n + 1) * 512],
                    start=(t == 0),
                    stop=(t == NJ - 1),
                )

        # ------------------------------------------------------------------
        # Tail: out = elu(T[0:F]/T[F]), 2 pipelined 512-chunks.
        # Reciprocal on the Scalar engine (both halves, one table load),
        # denominator broadcast via ones-column matmul, elu as
        # max(x,0) + min(exp(x)-1, 0) so the Act exp overlaps DVE.
        # ------------------------------------------------------------------
        ones_row = singles.tile([1, F_OUT], BF16)
        nc.vector.memset(ones_row[:, :], 1.0)
        zr_row = singles.tile([1, R], BF16)
        h_c = singles.tile([F_OUT, R], F32)
        ot = singles.tile([F_OUT, R], F32)
        ex_sb = singles.tile([F_OUT, R], F32)
        ob = singles.tile([F_OUT, R], BF16)
        for n in range(2):
            sl = slice(n * 512, (n + 1) * 512)
            _act_recip(nc, zr_row[:, sl], acc[n][F_OUT:FX, :])
            nc.vector.tensor_copy(h_c[:, sl], acc[n][0:F_OUT, :])
        for n in range(2):
            sl = slice(n * 512, (n + 1) * 512)
            zb_ps = misc_psum.tile([F_OUT, 512], F32, tag="mps")
            nc.tensor.matmul(
                zb_ps[:, :], ones_row[:, :], zr_row[:, sl], start=True, stop=True
            )
            nc.vector.tensor_tensor(ot[:, sl], h_c[:, sl], zb_ps[:, :], OP.mult)
            nc.scalar.activation(ex_sb[:, sl], ot[:, sl], AF.Exp)
            nc.vector.tensor_scalar(ot[:, sl], ot[:, sl], 0.0, None, OP.max)
            nc.vector.tensor_scalar(
                ex_sb[:, sl], ex_sb[:, sl], -1.0, 0.0, OP.add, OP.min
            )
            nc.vector.tensor_tensor(ob[:, sl], ot[:, sl], ex_sb[:, sl], OP.add)
            nc.sync.dma_start(out=out[:, sl], in_=ob[:, sl])

    if split_waits:
        _split_multi_waits(nc)
    return nc


_CACHED = {}


def _get_compiled():
    if "nc" not in _CACHED:
        _CACHED["nc"] = build_kernel()
    return _CACHED["nc"]


def kernel(t_input, o_input, W_t, W_o, a, adj, _trace=False):
    from concourse.bass_utils import run_bass_kernel_spmd

    t_input = np.asarray(t_input, dtype=np.float32)
    o_input = np.asarray(o_input, dtype=np.float32)
    W_t = np.asarray(W_t, dtype=np.float32)
    W_o = np.asarray(W_o, dtype=np.float32)
    a = np.asarray(a, dtype=np.float32)
    adj = np.asarray(adj)

    o_T = np.ascontiguousarray(o_input.T).astype(bf16)
    adj_b = adj.astype(bf16)
    w_to = np.ascontiguousarray(np.concatenate([W_t, W_o], axis=1))

    in_maps = []
    for m in range(N_CORES):
        rows = slice(m * R, (m + 1) * R)
        in_maps.append(
            {
                "t_T": np.ascontiguousarray(t_input[rows, :].T).astype(bf16),
                "o_T": o_T,
                "w_to": w_to,
                "a_vec": a,
                "adjT": np.ascontiguousarray(adj_b[rows, :].T),
            }
        )

    nc = _get_compiled()
    res = run_bass_kernel_spmd(
        nc, in_maps, core_ids=list(range(N_CORES)), trace=_trace
    )
    out = np.empty((N_T, F_OUT), dtype=np.float32)
    for m in range(N_CORES):
        out[m * R:(m + 1) * R, :] = res.results[m]["out"].T.astype(np.float32)
    if _trace:
        kernel.last_exec_time_ns = res.exec_time_ns
        kernel.last_results = res
    return out


# revision 41
# speedup vs baseline: 1.1112x; 1.0070x over previous
"""Trainium2 Bass kernel for the NodeAttentionLayer (GAT-style) problem.

Math (per reference.py):
    h_t = t_input @ W_t; h_o = o_input @ W_o
    s_t = h_t @ a[:F];  s_o = h_o @ a[F:]
    e[i,j]   = leaky_relu(s_t[i] + s_o[j], 0.2)
    att      = softmax(where(adj>0, e, -9e15), axis=1)
    out      = elu(att @ h_o)

Single-mask identity used on-device:
    exp(lrelu(y)) = exp(0.2 y) * max(exp(0.8 y), 1),   y = s_t[i] + s_o[j]
The i-side factor exp(0.2 s_t) cancels in the softmax ratio, so with
    q[j,i] = max(exp(0.8 s_t_i) * exp(s_o_j), exp(0.2 s_o_j))
           = exp(0.2 s_o_j) * max(exp(0.8 y), 1)
the (unnormalized, u2-cancelled) attention is m = q * adj and
    out[:,i] = elu( (W_ext^T m)[0:F,i] / (W_ext^T m)[F,i] ),  W_ext = [h_o | 1].
q is ONE dual-op tensor_scalar (mult,max with two per-partition scalar
columns) and m ONE tensor_tensor mult per j-tile — both on DVE in 2x/4x
modes.  One matmul pair per tile accumulates in PSUM over all 64 tiles
with a fixed 65-column stationary per tile.  GpSimd is untouched (its
SBUF port contention slows DVE ~4x).

h_t is never materialized: s_t = t_input @ (W_t a_t) is a matvec, and
s_o rides as a 65th projection column ([W_o | W_o a_o] moving operand).

Sharding: rows of t_input/adj (N_t) split across 8 cores; o replicated.
Kernel computes output TRANSPOSED ([F, rows]) per core; host transposes.
adj fed per-core as adj[rows,:].T in bf16 (0/1 -> lossless); t/o in bf16.
"""

import contextlib
import ctypes
import sys
import tempfile
import types

import ml_dtypes
import numpy as np

import concourse.bass as bass
import concourse.mybir as mybir
import concourse.tile as tile
from concourse.vector_clock import ScopedClock

bf16 = ml_dtypes.bfloat16

# ---------------------------------------------------------------------------
# Environment shims
# ---------------------------------------------------------------------------

def _patch_tile_drain():
    """walrus in this container allows only one sync-wait per sync-engine
    instruction; split the TileContext epilogue drain's waits onto
    individual nops."""
    if getattr(tile.TileContext, "_drain_patch_installed", False):
        return

    def _drain_and_barrier(self, tick_clock, wait_clock):
        nop_inst = self.nc.sync.nop(nofuse=True)
        wait_clock.add_sem_waits(
            nop_inst.ins, ScopedClock({None: tick_clock.global_clock})
        )
        ow = list(nop_inst.ins.sync_info.on_wait) if nop_inst.ins.sync_info else []
        if len(ow) > 1:
            nop_inst.ins.sync_info.on_wait = ow[:1]
            for w in ow[1:]:
                extra = self.nc.sync.nop(nofuse=True)
                if extra.ins.sync_info is None:
                    extra.ins.sync_info = mybir.SyncInfo(on_wait=[w], on_update=[])
                else:
                    extra.ins.sync_info.on_wait = [w]
        self.nc.sync.drain()
        self.nc.all_engine_barrier()
        popped = self.nc._tile_sem_poison_stack.pop()
        assert popped is self._sem_poison
        self.nc.clear_and_free_semaphores(list(self.sems.allocated().values()))
        self.nc.all_engine_barrier()

    tile.TileContext._drain_and_barrier = _drain_and_barrier
    tile.TileContext._drain_patch_installed = True


def _install_ntff_hook():
    """Provide antenv.axon_hooks (absent in this image) so trace=True works."""
    if "antenv.axon_hooks" in sys.modules:
        return
    import antenv

    state = {"hook": None}
    mod = types.ModuleType("antenv.axon_hooks")
    mod.set_axon_ntff_profile_hook = lambda h: state.__setitem__("hook", h)
    mod.get_axon_ntff_profile_hook = lambda: state["hook"]
    sys.modules["antenv.axon_hooks"] = mod
    antenv.axon_hooks = mod

    try:
        lib = ctypes.CDLL("/opt/axon/libaxon_pjrt.so")
    except OSError:
        return
    if not hasattr(lib, "axon_start_nrt_profile"):
        return
    lib.axon_start_nrt_profile.argtypes = [
        ctypes.POINTER(ctypes.c_int64),
        ctypes.c_size_t,
    ]
    lib.axon_start_nrt_profile.restype = ctypes.c_int64
    lib.axon_stop_nrt_profile.argtypes = [ctypes.c_char_p]
    lib.axon_stop_nrt_profile.restype = ctypes.c_int64

    @contextlib.contextmanager
    def _ntff_hook(output_dir, device_ids):
        import jax

        jax.devices()
        if device_ids:
            ids = (ctypes.c_int64 * len(device_ids))(*device_ids)
            rc = lib.axon_start_nrt_profile(ids, len(device_ids))
        else:
            rc = lib.axon_start_nrt_profile(None, 0)
        if rc != 0:
            raise RuntimeError(f"axon_start_nrt_profile rc={rc}")
        try:
            yield
        finally:
            n = lib.axon_stop_nrt_profile(str(output_dir).encode())
            print(f"profile: {n} file(s) written to {output_dir}", file=sys.stderr)

    state["hook"] = _ntff_hook


_patch_tile_drain()
_install_ntff_hook()


def _split_multi_waits(nc):
    """walrus here accepts at most ONE sync-wait per instruction; hoist extra
    waits onto same-engine nops inserted immediately before."""
    import bass_rust

    k = 0
    for f in nc.m.functions:
        for blk in f.blocks:
            insts = blk.instructions
            out = []
            changed = False
            for inst in insts:
                si = inst.sync_info
                ow = list(si.on_wait) if si is not None else []
                if len(ow) > 1:
                    for w in ow[:-1]:
                        nop = bass_rust.InstNoOp(
                            name=f"waitsplit-{k}", engine=inst.engine
                        )
                        k += 1
                        nop.sync_info = mybir.SyncInfo(on_wait=[w], on_update=[])
                        out.append(nop)
                    si.on_wait = [ow[-1]]
                    changed = True
                out.append(inst)
            if changed:
                blk.instructions = out

# ---------------------------------------------------------------------------
# Problem constants (hardcoded per spec)
# ---------------------------------------------------------------------------
N_T, N_O, F_IN, F_OUT = 8192, 8192, 256, 64
N_CORES = 8
R = N_T // N_CORES            # rows (i) per core = 1024
NJ = N_O // 128               # j tiles of 128 = 64
KC = F_IN // 128              # contraction chunks for projections = 2
GROUP = 16                    # j-tiles per setup group
NG = NJ // GROUP              # setup groups = 4
FX = F_OUT + 1                # 65 (h_o columns + ones/denominator column)
F32 = mybir.dt.float32
BF16 = mybir.dt.bfloat16
AF = mybir.ActivationFunctionType
OP = mybir.AluOpType


def _ap(tensor, offset, ap):
    return bass.AP(tensor=tensor, offset=offset, ap=ap)


def _act_recip(nc, out, in_):
    """Raw InstActivation(Reciprocal) — table-interp reciprocal on the Scalar
    engine.  bass.activation() refuses Reciprocal on accuracy grounds; our
    denominators are well-conditioned sums of positive terms and the output
    gate is 2e-2, so the table version is fine (validated vs reference)."""
    eng = nc.scalar
    inputs = [eng.lower_ap(in_)]
    for v in (0.0, 1.0, 0.0):  # bias, scale, alpha
        inputs.append(mybir.ImmediateValue(dtype=mybir.dt.float32, value=v))
    return eng.add_instruction(
        mybir.InstActivation(
            name=eng.bass.get_next_instruction_name(),
            func=AF.Reciprocal,
            ins=inputs,
            outs=[eng.lower_ap(out)],
        )
    )


def build_kernel(split_waits=True):
    nc = bass.Bass("TRN2")

    t_T = nc.dram_tensor("t_T", [F_IN, R], BF16, kind="ExternalInput")
    o_T = nc.dram_tensor("o_T", [F_IN, N_O], BF16, kind="ExternalInput")
    w_to = nc.dram_tensor("w_to", [F_IN, 2 * F_OUT], F32, kind="ExternalInput")
    a_vec = nc.dram_tensor("a_vec", [2 * F_OUT, 1], F32, kind="ExternalInput")
    adjT = nc.dram_tensor("adjT", [N_O, R], BF16, kind="ExternalInput")
    out = nc.dram_tensor("out", [F_OUT, R], BF16, kind="ExternalOutput")

    with tile.TileContext(nc) as tc, contextlib.ExitStack() as ctx:
        adj_pool = ctx.enter_context(tc.tile_pool(name="adj", bufs=8))
        q_pool = ctx.enter_context(tc.tile_pool(name="q", bufs=6))
        m_pool = ctx.enter_context(tc.tile_pool(name="m", bufs=6))
        singles = ctx.enter_context(tc.tile_pool(name="singles", bufs=1))
        stage = ctx.enter_context(tc.tile_pool(name="stage", bufs=2))
        acc_psum = ctx.enter_context(tc.tile_pool(name="acc", bufs=1, space="PSUM"))
        misc_psum = ctx.enter_context(tc.tile_pool(name="mpsum", bufs=2, space="PSUM"))

        # ------------------------------------------------------------------
        # Weights + a-vector broadcasts (small loads issued from the
        # otherwise-idle GpSimd/Scalar sequencers so they don't queue behind
        # the big sync-engine DMA stream)
        # ------------------------------------------------------------------
        w_sb = singles.tile([128, KC, 2 * F_OUT], F32)
        nc.sync.dma_start(
            out=w_sb[:, :, :],
            in_=_ap(w_to, 0, [[2 * F_OUT, 128], [128 * 2 * F_OUT, KC], [1, 2 * F_OUT]]),
        )
        at_b = singles.tile([128, F_OUT], F32)
        ao_b = singles.tile([128, F_OUT], F32)
        nc.sync.dma_start(out=at_b[:, :], in_=_ap(a_vec, 0, [[0, 128], [1, F_OUT]]))
        nc.scalar.dma_start(
            out=ao_b[:, :], in_=_ap(a_vec, F_OUT, [[0, 128], [1, F_OUT]])
        )
        t_sb = singles.tile([128, KC, R], BF16)
        nc.scalar.dma_start(
            out=t_sb[:, :, :], in_=_ap(t_T, 0, [[R, 128], [128 * R, KC], [1, R]])
        )

        # wta = W_t @ a_t replicated to a [128] stationary block (so the s_t
        # matmul broadcasts s_t to all partitions); woa = W_o @ a_o (65th col)
        prod_t = stage.tile([128, KC, F_OUT], F32, tag="prod")
        nc.vector.tensor_tensor(
            prod_t[:, :, :],
            w_sb[:, :, 0:F_OUT],
            _ap(at_b[:, :].tensor, at_b[:, :].offset,
                [at_b[:, :].ap[0], [0, KC], [1, F_OUT]]),
            OP.mult,
        )
        red_t = stage.tile([128, KC], F32, tag="red")
        nc.vector.tensor_reduce(red_t[:, :], prod_t[:, :, :], mybir.AxisListType.X, OP.add)
        wta_rep = singles.tile([128, KC, 128], BF16)
        nc.vector.tensor_copy(
            wta_rep[:, :, :],
            _ap(red_t[:, :].tensor, red_t[:, :].offset,
                [red_t[:, :].ap[0], [1, KC], [0, 128]]),
        )

        prod_o = stage.tile([128, KC, F_OUT], F32, tag="prod")
        nc.vector.tensor_tensor(
            prod_o[:, :, :],
            w_sb[:, :, F_OUT:2 * F_OUT],
            _ap(ao_b[:, :].tensor, ao_b[:, :].offset,
                [ao_b[:, :].ap[0], [0, KC], [1, F_OUT]]),
            OP.mult,
        )
        woa_f = stage.tile([128, KC], F32, tag="red")
        nc.vector.tensor_reduce(woa_f[:, :], prod_o[:, :, :], mybir.AxisListType.X, OP.add)

        # moving operand for the o-projection: [W_o | W_o a_o] in bf16
        woe = singles.tile([128, KC, FX], BF16)
        nc.vector.tensor_copy(woe[:, :, 0:F_OUT], w_sb[:, :, F_OUT:2 * F_OUT])
        nc.vector.tensor_copy(woe[:, :, F_OUT], woa_f[:, :])

        # ------------------------------------------------------------------
        # o side, graded groups (small first group so the main loop starts
        # early): h_o_ext = [h_o | s_o] -> hoe=[h_o|1], v1, v2
        # ------------------------------------------------------------------
        GROUPS = [(0, 8), (8, 8), (16, 8), (24, 8), (32, 8), (40, 8), (48, 8), (56, 8)]
        tile2grp = {}
        for gi, (t0, nt) in enumerate(GROUPS):
            for u in range(nt):
                tile2grp[t0 + u] = (gi, u)

        hoe, v1g, v2g = [], [], []

        def o_group(gi):
            t0, nt = GROUPS[gi]
            cols = nt * 128
            o_sb = stage.tile([128, KC, nt * 128], BF16, tag="osb")
            for c in range(KC):
                for h0 in range(0, cols, 1024):
                    h1 = min(h0 + 1024, cols)
                    nc.sync.dma_start(
                        out=o_sb[:, c, h0:h1],
                        in_=o_T[c * 128:(c + 1) * 128,
                                t0 * 128 + h0:t0 * 128 + h1],
                    )
            hoe_g = singles.tile([128, nt, FX], BF16, tag=f"hoe{gi}")
            v1_g = singles.tile([128, nt], F32, tag=f"v1{gi}")
            v2_g = singles.tile([128, nt], F32, tag=f"v2{gi}")
            for b in range(nt // 4):
                ho_ps = misc_psum.tile([128, 4, FX], F32, tag="mps")
                for s in range(4):
                    j0 = (b * 4 + s) * 128
                    for c in range(KC):
                        nc.tensor.matmul(
                            ho_ps[:, s, :],
                            o_sb[:, c, j0:j0 + 128],
                            woe[:, c, :],
                            start=(c == 0),
                            stop=(c == KC - 1),
                        )
                sl = slice(b * 4, b * 4 + 4)
                nc.scalar.activation(v1_g[:, sl], ho_ps[:, :, F_OUT], AF.Exp)
                nc.scalar.activation(
                    v2_g[:, sl], ho_ps[:, :, F_OUT], AF.Exp, scale=0.2
                )
                nc.scalar.activation(hoe_g[:, sl, 0:F_OUT], ho_ps[:, :, 0:F_OUT], AF.Copy)
                nc.vector.memset(hoe_g[:, sl, F_OUT], 1.0)
            hoe.append(hoe_g)
            v1g.append(v1_g)
            v2g.append(v2_g)

        o_group(0)
        o_group(1)  # tiles 0-15 pre-loop; rest JIT

        # ------------------------------------------------------------------
        # t side: s_t broadcast to all 128 partitions directly by the matmul
        # (stationary = wta replicated 128x); u8 = exp(0.8 s_t) on Act.
        # ------------------------------------------------------------------
        u8_b = singles.tile([128, R], BF16)
        with tc.high_priority():
            for n in range(R // 512):
                sl = slice(n * 512, (n + 1) * 512)
                st_ps = misc_psum.tile([128, 512], F32, tag="stps")
                for c in range(KC):
                    nc.tensor.matmul(
                        st_ps[:, :],
                        wta_rep[:, c, :],
                        t_sb[:, c, sl],
                        start=(c == 0),
                        stop=(c == KC - 1),
                    )
                nc.scalar.activation(u8_b[:, sl], st_ps[:, :], AF.Exp, scale=0.8)

        o_group(1)

        # ------------------------------------------------------------------
        # Main loop over j tiles: q = max(u8*v1, v2); m = q*adj; accumulate
        # adj DMA'd in 4-tile batches (fewer descriptors/semaphores).
        # ------------------------------------------------------------------
        acc = [
            acc_psum.tile([FX, 512], F32, tag=f"acc{n}", name=f"acc{n}")
            for n in range(2)
        ]
        for t in range(NJ):
            g, u = tile2grp[t]
            # emit later o-group setup just-in-time so its DMA doesn't
            # compete with the adj stream at loop start
            jit = {2: 2, 8: 3, 16: 4, 24: 5, 32: 6, 40: 7}
            if t in jit:
                o_group(jit[t])
            adj_t = adj_pool.tile([128, R], BF16)
            nc.sync.dma_start(out=adj_t[:, :], in_=adjT[t * 128:(t + 1) * 128, :])
            q_t = q_pool.tile([128, R], BF16)
            nc.vector.tensor_scalar(
                q_t[:, :], u8_b[:, :], v1g[g][:, u:u + 1], v2g[g][:, u:u + 1],
                OP.mult, OP.max,
            )
            m_t = m_pool.tile([128, R], BF16)
            nc.vector.tensor_tensor(m_t[:, :], q_t[:, :], adj_t[:, :], OP.mult)
            for n in range(2):
                nc.tensor.matmul(
                    acc[n][:, :],
                    hoe[g][:, u, :],
                    m_t[:, n * 512:(n + 1) * 512],
                    start=(t == 0),
                    stop=(t == NJ - 1),
                )

        # ------------------------------------------------------------------
        # Tail: out = elu(T[0:F]/T[F]), 2 pipelined 512-chunks.
        # Reciprocal on the Scalar engine (both halves, one table load),
        # denominator broadcast via ones-column matmul, elu as
        # max(x,0) + min(exp(x)-1, 0) so the Act exp overlaps DVE.
        # ------------------------------------------------------------------
        ones_row = singles.tile([1, F_OUT], BF16)
        nc.vector.memset(ones_row[:, :], 1.0)
        zr_row = singles.tile([1, R], BF16)
        h_c = singles.tile([F_OUT, R], F32)
        ot = singles.tile([F_OUT, R], F32)
        ex_sb = singles.tile([F_OUT, R], F32)
        ob = singles.tile([F_OUT, R], BF16)
        for n in range(2):
            sl = slice(n * 512, (n + 1) * 512)
            _act_recip(nc, zr_row[:, sl], acc[n][F_OUT:FX, :])
            nc.vector.tensor_copy(h_c[:, sl], acc[n][0:F_OUT, :])
        for n in range(2):
            sl = slice(n * 512, (n + 1) * 512)
            zb_ps = misc_psum.tile([F_OUT, 512], F32, tag="mps")
            nc.tensor.matmul(
                zb_ps[:, :], ones_row[:, :], zr_row[:, sl], start=True, stop=True
            )
            nc.vector.tensor_tensor(ot[:, sl], h_c[:, sl], zb_ps[:, :], OP.mult)
            nc.scalar.activation(ex_sb[:, sl], ot[:, sl], AF.Exp)
            nc.vector.tensor_scalar(ot[:, sl], ot[:, sl], 0.0, None, OP.max)
            nc.vector.tensor_scalar(
                ex_sb[:, sl], ex_sb[:, sl], -1.0, 0.0, OP.add, OP.min
            )
            nc.vector.tensor_tensor(ob[:, sl], ot[:, sl], ex_sb[:, sl], OP.add)
            nc.sync.dma_start(out=out[:, sl], in_=ob[:, sl])

    if split_waits:
        _split_multi_waits(nc)
    return nc


_CACHED = {}


def _get_compiled():
    if "nc" not in _CACHED:
        _CACHED["nc"] = build_kernel()
    return _CACHED["nc"]


def kernel(t_input, o_input, W_t, W_o, a, adj, _trace=False):
    from concourse.bass_utils import run_bass_kernel_spmd

    t_input = np.asarray(t_input, dtype=np.float32)
    o_input = np.asarray(o_input, dtype=np.float32)
    W_t = np.asarray(W_t, dtype=np.float32)
    W_o = np.asarray(W_o, dtype=np.float32)
    a = np.asarray(a, dtype=np.float32)
    adj = np.asarray(adj)

    o_T = np.ascontiguousarray(o_input.T).astype(bf16)
    adj_b = adj.astype(bf16)
    w_to = np.ascontiguousarray(np.concatenate([W_t, W_o], axis=1))

    in_maps = []
    for m in range(N_CORES):
        rows = slice(m * R, (m + 1) * R)
        in_maps.append(
            {
                "t_T": np.ascontiguousarray(t_input[rows, :].T).astype(bf16),
                "o_T": o_T,
                "w_to": w_to,
                "a_vec": a,
                "adjT": np.ascontiguousarray(adj_b[rows, :].T),
            }
        )

    nc = _get_compiled()
    res = run_bass_kernel_spmd(
        nc, in_maps, core_ids=list(range(N_CORES)), trace=_trace
    )
    out = np.empty((N_T, F_OUT), dtype=np.float32)
    for m in range(N_CORES):
        out[m * R:(m + 1) * R, :] = res.results[m]["out"].T.astype(np.float32)
    if _trace:
        kernel.last_exec_time_ns = res.exec_time_ns
        kernel.last_results = res
    return out


# revision 42
# speedup vs baseline: 1.1115x; 1.0003x over previous
"""Trainium2 Bass kernel for the NodeAttentionLayer (GAT-style) problem.

Math (per reference.py):
    h_t = t_input @ W_t; h_o = o_input @ W_o
    s_t = h_t @ a[:F];  s_o = h_o @ a[F:]
    e[i,j]   = leaky_relu(s_t[i] + s_o[j], 0.2)
    att      = softmax(where(adj>0, e, -9e15), axis=1)
    out      = elu(att @ h_o)

Single-mask identity used on-device:
    exp(lrelu(y)) = exp(0.2 y) * max(exp(0.8 y), 1),   y = s_t[i] + s_o[j]
The i-side factor exp(0.2 s_t) cancels in the softmax ratio, so with
    q[j,i] = max(exp(0.8 s_t_i) * exp(s_o_j), exp(0.2 s_o_j))
           = exp(0.2 s_o_j) * max(exp(0.8 y), 1)
the (unnormalized, u2-cancelled) attention is m = q * adj and
    out[:,i] = elu( (W_ext^T m)[0:F,i] / (W_ext^T m)[F,i] ),  W_ext = [h_o | 1].
q is ONE dual-op tensor_scalar (mult,max with two per-partition scalar
columns) and m ONE tensor_tensor mult per j-tile — both on DVE in 2x/4x
modes.  One matmul pair per tile accumulates in PSUM over all 64 tiles
with a fixed 65-column stationary per tile.  GpSimd is untouched (its
SBUF port contention slows DVE ~4x).

h_t is never materialized: s_t = t_input @ (W_t a_t) is a matvec, and
s_o rides as a 65th projection column ([W_o | W_o a_o] moving operand).

Sharding: rows of t_input/adj (N_t) split across 8 cores; o replicated.
Kernel computes output TRANSPOSED ([F, rows]) per core; host transposes.
adj fed per-core as adj[rows,:].T in bf16 (0/1 -> lossless); t/o in bf16.
"""

import contextlib
import ctypes
import sys
import tempfile
import types

import ml_dtypes
import numpy as np

import concourse.bass as bass
import concourse.mybir as mybir
import concourse.tile as tile
from concourse.vector_clock import ScopedClock

bf16 = ml_dtypes.bfloat16

# ---------------------------------------------------------------------------
# Environment shims
# ---------------------------------------------------------------------------

def _patch_tile_drain():
    """walrus in this container allows only one sync-wait per sync-engine
    instruction; split the TileContext epilogue drain's waits onto
    individual nops."""
    if getattr(tile.TileContext, "_drain_patch_installed", False):
        return

    def _drain_and_barrier(self, tick_clock, wait_clock):
        nop_inst = self.nc.sync.nop(nofuse=True)
        wait_clock.add_sem_waits(
            nop_inst.ins, ScopedClock({None: tick_clock.global_clock})
        )
        ow = list(nop_inst.ins.sync_info.on_wait) if nop_inst.ins.sync_info else []
        if len(ow) > 1:
            nop_inst.ins.sync_info.on_wait = ow[:1]
            for w in ow[1:]:
                extra = self.nc.sync.nop(nofuse=True)
                if extra.ins.sync_info is None:
                    extra.ins.sync_info = mybir.SyncInfo(on_wait=[w], on_update=[])
                else:
                    extra.ins.sync_info.on_wait = [w]
        self.nc.sync.drain()
        self.nc.all_engine_barrier()
        popped = self.nc._tile_sem_poison_stack.pop()
        assert popped is self._sem_poison
        self.nc.clear_and_free_semaphores(list(self.sems.allocated().values()))
        self.nc.all_engine_barrier()

    tile.TileContext._drain_and_barrier = _drain_and_barrier
    tile.TileContext._drain_patch_installed = True


def _install_ntff_hook():
    """Provide antenv.axon_hooks (absent in this image) so trace=True works."""
    if "antenv.axon_hooks" in sys.modules:
        return
    import antenv

    state = {"hook": None}
    mod = types.ModuleType("antenv.axon_hooks")
    mod.set_axon_ntff_profile_hook = lambda h: state.__setitem__("hook", h)
    mod.get_axon_ntff_profile_hook = lambda: state["hook"]
    sys.modules["antenv.axon_hooks"] = mod
    antenv.axon_hooks = mod

    try:
        lib = ctypes.CDLL("/opt/axon/libaxon_pjrt.so")
    except OSError:
        return
    if not hasattr(lib, "axon_start_nrt_profile"):
        return
    lib.axon_start_nrt_profile.argtypes = [
        ctypes.POINTER(ctypes.c_int64),
        ctypes.c_size_t,
    ]
    lib.axon_start_nrt_profile.restype = ctypes.c_int64
    lib.axon_stop_nrt_profile.argtypes = [ctypes.c_char_p]
    lib.axon_stop_nrt_profile.restype = ctypes.c_int64

    @contextlib.contextmanager
    def _ntff_hook(output_dir, device_ids):
        import jax

        jax.devices()
        if device_ids:
            ids = (ctypes.c_int64 * len(device_ids))(*device_ids)
            rc = lib.axon_start_nrt_profile(ids, len(device_ids))
        else:
            rc = lib.axon_start_nrt_profile(None, 0)
        if rc != 0:
            raise RuntimeError(f"axon_start_nrt_profile rc={rc}")
        try:
            yield
        finally:
            n = lib.axon_stop_nrt_profile(str(output_dir).encode())
            print(f"profile: {n} file(s) written to {output_dir}", file=sys.stderr)

    state["hook"] = _ntff_hook


_patch_tile_drain()
_install_ntff_hook()


def _split_multi_waits(nc):
    """walrus here accepts at most ONE sync-wait per instruction; hoist extra
    waits onto same-engine nops inserted immediately before."""
    import bass_rust

    k = 0
    for f in nc.m.functions:
        for blk in f.blocks:
            insts = blk.instructions
            out = []
            changed = False
            for inst in insts:
                si = inst.sync_info
                ow = list(si.on_wait) if si is not None else []
                if len(ow) > 1:
                    for w in ow[:-1]:
                        nop = bass_rust.InstNoOp(
                            name=f"waitsplit-{k}", engine=inst.engine
                        )
                        k += 1
                        nop.sync_info = mybir.SyncInfo(on_wait=[w], on_update=[])
                        out.append(nop)
                    si.on_wait = [ow[-1]]
                    changed = True
                out.append(inst)
            if changed:
                blk.instructions = out

# ---------------------------------------------------------------------------
# Problem constants (hardcoded per spec)
# ---------------------------------------------------------------------------
N_T, N_O, F_IN, F_OUT = 8192, 8192, 256, 64
N_CORES = 8
R = N_T // N_CORES            # rows (i) per core = 1024
NJ = N_O // 128               # j tiles of 128 = 64
KC = F_IN // 128              # contraction chunks for projections = 2
GROUP = 16                    # j-tiles per setup group
NG = NJ // GROUP              # setup groups = 4
FX = F_OUT + 1                # 65 (h_o columns + ones/denominator column)
F32 = mybir.dt.float32
BF16 = mybir.dt.bfloat16
AF = mybir.ActivationFunctionType
OP = mybir.AluOpType


def _ap(tensor, offset, ap):
    return bass.AP(tensor=tensor, offset=offset, ap=ap)


def _act_recip(nc, out, in_):
    """Raw InstActivation(Reciprocal) — table-interp reciprocal on the Scalar
    engine.  bass.activation() refuses Reciprocal on accuracy grounds; our
    denominators are well-conditioned sums of positive terms and the output
    gate is 2e-2, so the table version is fine (validated vs reference)."""
    eng = nc.scalar
    inputs = [eng.lower_ap(in_)]
    for v in (0.0, 1.0, 0.0):  # bias, scale, alpha
        inputs.append(mybir.ImmediateValue(dtype=mybir.dt.float32, value=v))
    return eng.add_instruction(
        mybir.InstActivation(
            name=eng.bass.get_next_instruction_name(),
            func=AF.Reciprocal,
            ins=inputs,
            outs=[eng.lower_ap(out)],
        )
    )


def build_kernel(split_waits=True):
    nc = bass.Bass("TRN2")

    t_T = nc.dram_tensor("t_T", [F_IN, R], BF16, kind="ExternalInput")
    o_T = nc.dram_tensor("o_T", [F_IN, N_O], BF16, kind="ExternalInput")
    w_to = nc.dram_tensor("w_to", [F_IN, 2 * F_OUT], F32, kind="ExternalInput")
    a_vec = nc.dram_tensor("a_vec", [2 * F_OUT, 1], F32, kind="ExternalInput")
    adjT = nc.dram_tensor("adjT", [N_O, R], BF16, kind="ExternalInput")
    out = nc.dram_tensor("out", [F_OUT, R], BF16, kind="ExternalOutput")

    with tile.TileContext(nc) as tc, contextlib.ExitStack() as ctx:
        adj_pool = ctx.enter_context(tc.tile_pool(name="adj", bufs=8))
        q_pool = ctx.enter_context(tc.tile_pool(name="q", bufs=6))
        m_pool = ctx.enter_context(tc.tile_pool(name="m", bufs=6))
        singles = ctx.enter_context(tc.tile_pool(name="singles", bufs=1))
        stage = ctx.enter_context(tc.tile_pool(name="stage", bufs=2))
        acc_psum = ctx.enter_context(tc.tile_pool(name="acc", bufs=1, space="PSUM"))
        misc_psum = ctx.enter_context(tc.tile_pool(name="mpsum", bufs=2, space="PSUM"))

        # ------------------------------------------------------------------
        # Weights + a-vector broadcasts (small loads issued from the
        # otherwise-idle GpSimd/Scalar sequencers so they don't queue behind
        # the big sync-engine DMA stream)
        # ------------------------------------------------------------------
        w_sb = singles.tile([128, KC, 2 * F_OUT], F32)
        nc.sync.dma_start(
            out=w_sb[:, :, :],
            in_=_ap(w_to, 0, [[2 * F_OUT, 128], [128 * 2 * F_OUT, KC], [1, 2 * F_OUT]]),
        )
        at_b = singles.tile([128, F_OUT], F32)
        ao_b = singles.tile([128, F_OUT], F32)
        nc.sync.dma_start(out=at_b[:, :], in_=_ap(a_vec, 0, [[0, 128], [1, F_OUT]]))
        nc.scalar.dma_start(
            out=ao_b[:, :], in_=_ap(a_vec, F_OUT, [[0, 128], [1, F_OUT]])
        )
        t_sb = singles.tile([128, KC, R], BF16)
        nc.scalar.dma_start(
            out=t_sb[:, :, :], in_=_ap(t_T, 0, [[R, 128], [128 * R, KC], [1, R]])
        )

        # wta = W_t @ a_t replicated to a [128] stationary block (so the s_t
        # matmul broadcasts s_t to all partitions); woa = W_o @ a_o (65th col)
        prod_t = stage.tile([128, KC, F_OUT], F32, tag="prod")
        nc.vector.tensor_tensor(
            prod_t[:, :, :],
            w_sb[:, :, 0:F_OUT],
            _ap(at_b[:, :].tensor, at_b[:, :].offset,
                [at_b[:, :].ap[0], [0, KC], [1, F_OUT]]),
            OP.mult,
        )
        red_t = stage.tile([128, KC], F32, tag="red")
        nc.vector.tensor_reduce(red_t[:, :], prod_t[:, :, :], mybir.AxisListType.X, OP.add)
        wta_rep = singles.tile([128, KC, 128], BF16)
        nc.vector.tensor_copy(
            wta_rep[:, :, :],
            _ap(red_t[:, :].tensor, red_t[:, :].offset,
                [red_t[:, :].ap[0], [1, KC], [0, 128]]),
        )

        prod_o = stage.tile([128, KC, F_OUT], F32, tag="prod")
        nc.vector.tensor_tensor(
            prod_o[:, :, :],
            w_sb[:, :, F_OUT:2 * F_OUT],
            _ap(ao_b[:, :].tensor, ao_b[:, :].offset,
                [ao_b[:, :].ap[0], [0, KC], [1, F_OUT]]),
            OP.mult,
        )
        woa_f = stage.tile([128, KC], F32, tag="red")
        nc.vector.tensor_reduce(woa_f[:, :], prod_o[:, :, :], mybir.AxisListType.X, OP.add)

        # moving operand for the o-projection: [W_o | W_o a_o] in bf16
        woe = singles.tile([128, KC, FX], BF16)
        nc.vector.tensor_copy(woe[:, :, 0:F_OUT], w_sb[:, :, F_OUT:2 * F_OUT])
        nc.vector.tensor_copy(woe[:, :, F_OUT], woa_f[:, :])

        # ------------------------------------------------------------------
        # o side, graded groups (small first group so the main loop starts
        # early): h_o_ext = [h_o | s_o] -> hoe=[h_o|1], v1, v2
        # ------------------------------------------------------------------
        GROUPS = [(0, 8), (8, 8), (16, 8), (24, 8), (32, 8), (40, 8), (48, 8), (56, 8)]
        tile2grp = {}
        for gi, (t0, nt) in enumerate(GROUPS):
            for u in range(nt):
                tile2grp[t0 + u] = (gi, u)

        hoe, v1g, v2g = [], [], []

        def o_group(gi):
            t0, nt = GROUPS[gi]
            cols = nt * 128
            o_sb = stage.tile([128, KC, nt * 128], BF16, tag="osb")
            for c in range(KC):
                for h0 in range(0, cols, 1024):
                    h1 = min(h0 + 1024, cols)
                    nc.sync.dma_start(
                        out=o_sb[:, c, h0:h1],
                        in_=o_T[c * 128:(c + 1) * 128,
                                t0 * 128 + h0:t0 * 128 + h1],
                    )
            hoe_g = singles.tile([128, nt, FX], BF16, tag=f"hoe{gi}")
            v1_g = singles.tile([128, nt], F32, tag=f"v1{gi}")
            v2_g = singles.tile([128, nt], F32, tag=f"v2{gi}")
            for b in range(nt // 4):
                ho_ps = misc_psum.tile([128, 4, FX], F32, tag="mps")
                for s in range(4):
                    j0 = (b * 4 + s) * 128
                    for c in range(KC):
                        nc.tensor.matmul(
                            ho_ps[:, s, :],
                            o_sb[:, c, j0:j0 + 128],
                            woe[:, c, :],
                            start=(c == 0),
                            stop=(c == KC - 1),
                        )
                sl = slice(b * 4, b * 4 + 4)
                nc.scalar.activation(v1_g[:, sl], ho_ps[:, :, F_OUT], AF.Exp)
                nc.scalar.activation(
                    v2_g[:, sl], ho_ps[:, :, F_OUT], AF.Exp, scale=0.2
                )
                nc.scalar.activation(hoe_g[:, sl, 0:F_OUT], ho_ps[:, :, 0:F_OUT], AF.Copy)
                nc.vector.memset(hoe_g[:, sl, F_OUT], 1.0)
            hoe.append(hoe_g)
            v1g.append(v1_g)
            v2g.append(v2_g)

        o_group(0)  # tiles 0-7 pre-loop; rest JIT

        # ------------------------------------------------------------------
        # t side: s_t broadcast to all 128 partitions directly by the matmul
        # (stationary = wta replicated 128x); u8 = exp(0.8 s_t) on Act.
        # ------------------------------------------------------------------
        u8_b = singles.tile([128, R], BF16)
        with tc.high_priority():
            for n in range(R // 512):
                sl = slice(n * 512, (n + 1) * 512)
                st_ps = misc_psum.tile([128, 512], F32, tag="stps")
                for c in range(KC):
                    nc.tensor.matmul(
                        st_ps[:, :],
                        wta_rep[:, c, :],
                        t_sb[:, c, sl],
                        start=(c == 0),
                        stop=(c == KC - 1),
                    )
                nc.scalar.activation(u8_b[:, sl], st_ps[:, :], AF.Exp, scale=0.8)

        o_group(1)

        # ------------------------------------------------------------------
        # Main loop over j tiles: q = max(u8*v1, v2); m = q*adj; accumulate
        # adj DMA'd in 4-tile batches (fewer descriptors/semaphores).
        # ------------------------------------------------------------------
        acc = [
            acc_psum.tile([FX, 512], F32, tag=f"acc{n}", name=f"acc{n}")
            for n in range(2)
        ]
        for t in range(NJ):
            g, u = tile2grp[t]
            # emit later o-group setup just-in-time so its DMA doesn't
            # compete with the adj stream at loop start
            jit = {0: 1, 4: 2, 10: 3, 18: 4, 26: 5, 34: 6, 42: 7}
            if t in jit:
                o_group(jit[t])
            adj_t = adj_pool.tile([128, R], BF16)
            nc.sync.dma_start(out=adj_t[:, :], in_=adjT[t * 128:(t + 1) * 128, :])
            q_t = q_pool.tile([128, R], BF16)
            nc.vector.tensor_scalar(
                q_t[:, :], u8_b[:, :], v1g[g][:, u:u + 1], v2g[g][:, u:u + 1],
                OP.mult, OP.max,
            )
            m_t = m_pool.tile([128, R], BF16)
            nc.vector.tensor_tensor(m_t[:, :], q_t[:, :], adj_t[:, :], OP.mult)
            for n in range(2):
                nc.tensor.matmul(
                    acc[n][:, :],
                    hoe[g][:, u, :],
                    m_t[:, n * 512:(n + 1) * 512],
                    start=(t == 0),
                    stop=(t == NJ - 1),
                )

        # ------------------------------------------------------------------
        # Tail: out = elu(T[0:F]/T[F]), 2 pipelined 512-chunks.
        # Reciprocal on the Scalar engine (both halves, one table load),
        # denominator broadcast via ones-column matmul, elu as
        # max(x,0) + min(exp(x)-1, 0) so the Act exp overlaps DVE.
        # ------------------------------------------------------------------
        ones_row = singles.tile([1, F_OUT], BF16)
        nc.vector.memset(ones_row[:, :], 1.0)
        zr_row = singles.tile([1, R], BF16)
        h_c = singles.tile([F_OUT, R], F32)
        ot = singles.tile([F_OUT, R], F32)
        ex_sb = singles.tile([F_OUT, R], F32)
        ob = singles.tile([F_OUT, R], BF16)
        for n in range(2):
            sl = slice(n * 512, (n + 1) * 512)
            _act_recip(nc, zr_row[:, sl], acc[n][F_OUT:FX, :])
            nc.vector.tensor_copy(h_c[:, sl], acc[n][0:F_OUT, :])
        for n in range(2):
            sl = slice(n * 512, (n + 1) * 512)
            zb_ps = misc_psum.tile([F_OUT, 512], F32, tag="mps")
            nc.tensor.matmul(
                zb_ps[:, :], ones_row[:, :], zr_row[:, sl], start=True, stop=True
            )
            nc.vector.tensor_tensor(ot[:, sl], h_c[:, sl], zb_ps[:, :], OP.mult)
            nc.scalar.activation(ex_sb[:, sl], ot[:, sl], AF.Exp)
            nc.vector.tensor_scalar(ot[:, sl], ot[:, sl], 0.0, None, OP.max)
            nc.vector.tensor_scalar(
                ex_sb[:, sl], ex_sb[:, sl], -1.0, 0.0, OP.add, OP.min
            )
            nc.vector.tensor_tensor(ob[:, sl], ot[:, sl], ex_sb[:, sl], OP.add)
            nc.sync.dma_start(out=out[:, sl], in_=ob[:, sl])

    if split_waits:
        _split_multi_waits(nc)
    return nc


_CACHED = {}


def _get_compiled():
    if "nc" not in _CACHED:
        _CACHED["nc"] = build_kernel()
    return _CACHED["nc"]


def kernel(t_input, o_input, W_t, W_o, a, adj, _trace=False):
    from concourse.bass_utils import run_bass_kernel_spmd

    t_input = np.asarray(t_input, dtype=np.float32)
    o_input = np.asarray(o_input, dtype=np.float32)
    W_t = np.asarray(W_t, dtype=np.float32)
    W_o = np.asarray(W_o, dtype=np.float32)
    a = np.asarray(a, dtype=np.float32)
    adj = np.asarray(adj)

    o_T = np.ascontiguousarray(o_input.T).astype(bf16)
    adj_b = adj.astype(bf16)
    w_to = np.ascontiguousarray(np.concatenate([W_t, W_o], axis=1))

    in_maps = []
    for m in range(N_CORES):
        rows = slice(m * R, (m + 1) * R)
        in_maps.append(
            {
                "t_T": np.ascontiguousarray(t_input[rows, :].T).astype(bf16),
                "o_T": o_T,
                "w_to": w_to,
                "a_vec": a,
                "adjT": np.ascontiguousarray(adj_b[rows, :].T),
            }
        )

    nc = _get_compiled()
    res = run_bass_kernel_spmd(
        nc, in_maps, core_ids=list(range(N_CORES)), trace=_trace
    )
    out = np.empty((N_T, F_OUT), dtype=np.float32)
    for m in range(N_CORES):
        out[m * R:(m + 1) * R, :] = res.results[m]["out"].T.astype(np.float32)
    if _trace:
        kernel.last_exec_time_ns = res.exec_time_ns
        kernel.last_results = res
    return out
